# revision 24
# baseline (speedup 1.0000x reference)
"""RNN-T joint network kernel for 8 Trainium2 NeuronCores.

out[b,t,u,c] = (enc[b,t,:] @ W[:, :D].T)[c] + (dec[b,u,:] @ W[:, D:].T)[c]

Sharding: data-parallel over (b, t-half): core i -> b = i//2, t-slab
[(i%2)*128, (i%2+1)*128).  Each core holds the full W, computes its
(128, 64, 1024) output slab (32 MB) and DMAs it out.  The output DMA
(32 MB/core at ~355-400 GB/s) is the roofline; everything else hides
under or ahead of it.

Host-side prep (part of the sharding/layout strategy):
  - W, enc, dec are pre-transposed so the contraction dim D sits on
    SBUF partitions with no on-chip transposes.
  - Each operand is split into exact bf16 hi+lo halves; on-chip GEMMs
    run 3 bf16 passes (hi*hi + hi*lo + lo*hi, error ~2^-18) which is
    both faster than fp32 matmul (4 passes internally) and lets the
    hi-pass start while the lo half is still streaming in.

Per-core dataflow:
  1. PE warm-up matmuls open the HAM clock gate during the input DMAs.
  2. 3-pass GEMMs -> dec_proj (PSUM), split into bf16 hi+lo directly;
     then enc_proj (128,1024) fp32 in SBUF.
  3. For each u: two accumulating K=128 bf16 selector matmuls broadcast
     dec_proj[u,:] across all 128 partitions into PSUM (sel[k,u,m]=k==u,
     built on the idle GpSimd); DVE adds enc_proj; groups of u's form
     contiguous 0.5-4 MB DMAs out (small first for an early first byte,
     large in steady state, small at the end for a short flush).
"""

import sys

import numpy as np

for _p in ("/opt/trn_rl_repo",):
    if _p not in sys.path:
        sys.path.insert(0, _p)

B, T, U, D, C = 4, 256, 64, 512, 1024
TSH = T // 2  # t-slab per core
NCORES = 8

_CACHE = {}


def _build_bass():
    import concourse.mybir as mybir
    from concourse import bacc
    from concourse.bass import ds
    from concourse.tile import TileContext

    f32 = mybir.dt.float32
    bf16 = mybir.dt.bfloat16
    add = mybir.AluOpType.add

    nc = bacc.Bacc("TRN2", target_bir_lowering=False, debug=False)
    dect_d = nc.declare_dram_parameter("dect", [2, D, U], bf16, isOutput=False)
    wtd_d = nc.declare_dram_parameter("wtd", [2, D, C], bf16, isOutput=False)
    enct_d = nc.declare_dram_parameter("enct", [2, D, TSH], bf16, isOutput=False)
    wte_d = nc.declare_dram_parameter("wte", [2, D, C], bf16, isOutput=False)
    o_d = nc.declare_dram_parameter("o", [TSH, U, C], f32, isOutput=True)

    with TileContext(nc) as tc:
        with (
            tc.tile_pool(name="const", bufs=1) as cpool,
            tc.tile_pool(name="outp", bufs=3) as opool,
        ):
            # sel[k, u, m] = 1.0 if k == u else 0.0 (k on partitions; rows
            # U..127 all zero so the selector matmuls are K=128 full-array
            # ops, which keeps the PE HAM clock warm). Built on the
            # otherwise-idle GpSimd to keep the DMA queues free for W.
            sel = cpool.tile([128, U, 128], bf16)
            nc.gpsimd.memset(sel[:], 0.0)
            nc.gpsimd.affine_select(
                out=sel[:],
                in_=sel[:],
                compare_op=mybir.AluOpType.not_equal,
                fill=1.0,
                base=0,
                pattern=[[-1, U], [0, 128]],
                channel_multiplier=1,
            )

            # ---- loads; dec side first (its chain to the first selector
            # PSUM is longer), hi halves before lo halves ----
            # layouts: xT[p, j, n] = x[hilo][j*128+p, n]
            decT = cpool.tile([128, 2, 4, U], bf16)
            wTd = cpool.tile([128, 2, 4, 1024], bf16)
            encT = cpool.tile([128, 2, 4, TSH], bf16)
            wTe = cpool.tile([128, 2, 4, 1024], bf16)
            dect_r = dect_d.rearrange("s (j p) u -> p s j u", p=128)
            wtd_r = wtd_d.rearrange("s (j p) c -> p s j c", p=128)
            enct_r = enct_d.rearrange("s (j p) t -> p s j t", p=128)
            wte_r = wte_d.rearrange("s (j p) c -> p s j c", p=128)
            nc.sync.dma_start(out=decT[:, 0], in_=dect_r[:, 0])
            nc.sync.dma_start(out=decT[:, 1], in_=dect_r[:, 1])
            for j in range(4):
                nc.sync.dma_start(out=wTd[:, 0, j, :], in_=wtd_r[:, 0, j, :])
            for j in range(4):
                nc.sync.dma_start(out=wTd[:, 1, j, :], in_=wtd_r[:, 1, j, :])
            nc.sync.dma_start(out=encT[:, 0], in_=enct_r[:, 0])
            nc.sync.dma_start(out=encT[:, 1], in_=enct_r[:, 1])
            for j in range(4):
                nc.sync.dma_start(out=wTe[:, 0, j, :], in_=wte_r[:, 0, j, :])
            for j in range(4):
                nc.sync.dma_start(out=wTe[:, 1, j, :], in_=wte_r[:, 1, j, :])

            enc_proj = cpool.tile([TSH, C], f32)
            # dec_proj = dec_hi + dec_lo, both bf16 (exact split to ~2^-18);
            # rows U..127 zero so K=128 matmuls pick up nothing from them.
            dec_hi = cpool.tile([128, C], bf16)
            dec_lo = cpool.tile([128, C], bf16)
            nc.vector.memset(dec_hi[U:, :], 0.0)
            nc.vector.memset(dec_lo[U:, :], 0.0)

            # PE warm-up source tile (zeroed; content irrelevant).
            warm_a = cpool.tile([128, 512], bf16)
            nc.vector.memset(warm_a[:], 0.0)

            with tc.tile_pool(name="psS", bufs=2, space="PSUM") as ppool:
                # PE warm-up: dependency-free matmuls issued while the input
                # DMAs stream, so the HAM clock gate opens (1.2 -> 2.4 GHz)
                # before the projection matmuls run.  Results are discarded.
                # The dummy ScalarE copy pulls the one-time ACT_TABLE_LOAD
                # (~1.3us) off the enc_proj critical path.
                wp = ppool.tile([128, 512], f32, tag="warm")
                for _ in range(10):
                    nc.tensor.matmul(
                        wp[:], warm_a[:, :128], warm_a[:], start=True, stop=True
                    )
                nc.scalar.copy(out=warm_a[:1, :32], in_=wp[:1, :32])

                # 3-pass hi/lo projection: hi*hi (all dt) first so it can
                # run before the lo chunks land, then hi*lo + lo*hi.
                def proj3(pp, xT, wT, h, rows):
                    passes = [(0, 0), (0, 1), (1, 0)]
                    for i, (sx, sw) in enumerate(passes):
                        for dt in range(4):
                            nc.tensor.matmul(
                                pp[:rows],
                                xT[:, sx, dt, :rows],
                                wT[:, sw, dt, ds(h * 512, 512)],
                                start=(i == 0 and dt == 0),
                                stop=(i == 2 and dt == 3),
                            )

                for h in range(2):
                    pp = ppool.tile([TSH, 512], f32, tag="proj")
                    proj3(pp, decT, wTd, h, U)
                    # hi/lo split straight from PSUM (no fp32 staging copy):
                    # ACT casts to bf16, DVE computes the bf16 remainder.
                    nc.scalar.copy(out=dec_hi[:U, ds(h * 512, 512)], in_=pp[:U])
                    nc.vector.tensor_tensor(
                        out=dec_lo[:U, ds(h * 512, 512)],
                        in0=pp[:U],
                        in1=dec_hi[:U, ds(h * 512, 512)],
                        op=mybir.AluOpType.subtract,
                    )

                for h in range(2):
                    pp = ppool.tile([TSH, 512], f32, tag="proj")
                    proj3(pp, encT, wTe, h, TSH)
                    nc.scalar.copy(out=enc_proj[:, ds(h * 512, 512)], in_=pp[:])

            # ---- main loop over u ----
            # small at the start (early first output byte), big in the
            # middle (descriptor efficiency), small at the end (short flush)
            groups = [1, 1, 2, 4] + [8] * 5 + [4] * 3 + [2, 1, 1]
            assert sum(groups) == U
            with tc.tile_pool(name="psM", bufs=2, space="PSUM") as mpool:
                u0 = 0
                for gsz in groups:
                    ot = opool.tile([TSH, gsz, C], f32, tag="out")
                    for jp in range((gsz + 1) // 2):
                        uw = min(2, gsz - jp * 2)  # u's in this psum tile
                        pr = mpool.tile([TSH, 2, C], f32, tag="rep")
                        for j2 in range(uw):
                            u = u0 + jp * 2 + j2
                            for h in range(2):
                                nc.tensor.matmul(
                                    pr[:, j2, ds(h * 512, 512)],
                                    sel[:, u, :],
                                    dec_hi[:, ds(h * 512, 512)],
                                    start=True,
                                    stop=False,
                                )
                                nc.tensor.matmul(
                                    pr[:, j2, ds(h * 512, 512)],
                                    sel[:, u, :],
                                    dec_lo[:, ds(h * 512, 512)],
                                    start=False,
                                    stop=True,
                                )
                        nc.vector.tensor_tensor(
                            out=ot[:, ds(jp * 2, uw), :],
                            in0=pr[:, :uw, :],
                            in1=enc_proj[:, None, :].to_broadcast([TSH, uw, C]),
                            op=add,
                        )
                    nc.sync.dma_start(
                        out=o_d[:, ds(u0, gsz), :], in_=ot[:, :gsz, :]
                    )
                    u0 += gsz

    nc.compile()
    return nc


def _get_nc():
    if "nc" not in _CACHE:
        _CACHE["nc"] = _build_bass()
    return _CACHE["nc"]


def _hilo(x):
    """Split float32 array into exact bf16 hi + lo halves, stacked on a
    leading axis: x ~= hi + lo with |x - hi - lo| <~ 2^-18 |x|."""
    import ml_dtypes

    hi = x.astype(ml_dtypes.bfloat16)
    lo = (x - hi.astype(np.float32)).astype(ml_dtypes.bfloat16)
    return np.ascontiguousarray(np.stack([hi, lo]))


def _make_in_maps(encoder_outputs, decoder_outputs, W):
    enc = np.asarray(encoder_outputs, dtype=np.float32)
    dec = np.asarray(decoder_outputs, dtype=np.float32)
    w = np.asarray(W, dtype=np.float32)

    wte = _hilo(w[:, :D].T)  # (2, D, C)
    wtd = _hilo(w[:, D:].T)  # (2, D, C)

    in_maps = []
    for i in range(NCORES):
        b, th = i // 2, i % 2
        enct = _hilo(enc[b, th * TSH : (th + 1) * TSH].T)  # (2, D, TSH)
        dect = _hilo(dec[b].T)  # (2, D, U)
        in_maps.append({"enct": enct, "dect": dect, "wte": wte, "wtd": wtd})
    return in_maps


def _run(encoder_outputs, decoder_outputs, W, trace=False):
    from concourse.bass_utils import run_bass_kernel_spmd

    nc = _get_nc()
    in_maps = _make_in_maps(encoder_outputs, decoder_outputs, W)
    res = run_bass_kernel_spmd(nc, in_maps, list(range(NCORES)), trace=trace)
    out = np.empty((B, T, U, C), dtype=np.float32)
    for i in range(NCORES):
        b, th = i // 2, i % 2
        out[b, th * TSH : (th + 1) * TSH] = res.results[i]["o"]
    return out, res


def kernel(encoder_outputs, decoder_outputs, W):
    out, _ = _run(encoder_outputs, decoder_outputs, W)
    return out
